# revision 33
# baseline (speedup 1.0000x reference)
"""Trainium2 Bass kernel for nn_BatchRelationalWithoutLocationsModule.

Math (per batch b, from the reference):
  o = x[b].reshape(c, h*w).T          # [L, c], c=64, L=256
  A = o @ W0[:c]; B = o @ W0[c:]      # [L, 32]
  h1_{ij} = relu(A_j + B_i + b0); h2 = relu(h1 @ W1 + b1); h3 = relu(h2 @ W2 + b2)
  s = sum_{ij} h3; out = relu(s @ Wp + bp) @ Wo + bo
  (sum is over all L^2 ordered pairs)

Distribution: pure data parallel, batch 32 -> 4 per core on 8 NeuronCores.

Device layout per core: partitions = 4 groups x 32 hidden. Group g handles
pair-row i = 64g + t. AT4pb [128, 256] = (A^T + b0) replicated over the 4
groups (bf16); BT4 [128, 64] column t = B^T[:, 64g+t] stacked by group (bf16).
Per chunk (4 t-values):
  act1 (x4): h1[:, kL:(k+1)L] = relu(AT4pb + BT4[:, t])  on DVE
             (bf16 in/out -> 4x packed mode, 2 elem read+write per port-cycle)
  mm1 (x2, N=512, bf16): p1 = W1bd^T h1   (W1bd = blockdiag(W1 x4))
  act2: h2 = relu(p1 + b14)  [128, 1024] -- ACT or DVE per balance pattern
  mm2 (x2): p2 = W2bd^T h2
  act3: h3 = relu(p2 + b24) + accum_out pair-sums -- ACT or DVE per pattern
The PSUM-sourced act2/act3 ops run at 1 elem/lane/cycle on either engine
(ACT 1.2 GHz, DVE 0.96 GHz); the balance pattern splits them ~2:1.
Then per batch reduce the chunk sums, fold the 4 groups with a 0/1 matmul,
and run the tiny head MLP on-chip.
"""
import sys
sys.path.insert(0, "/opt/trn_rl_repo")
import numpy as np

B, C, HW, L, H, NG = 32, 64, 16, 256, 32, 4
N_CORES = 8
B_PER_CORE = B // N_CORES
TPB = L // NG          # 64 t-values per batch
TPC = 4                # t-values per chunk
NCH = TPB // TPC       # 16 chunks per batch

_CACHE = {}

# evac engine pattern: 'A' = Activation engine, 'D' = DVE. Applied to the
# global sequence of act2/act3 ops round-robin.
DEFAULT_CFG = dict(
    evac_pattern=None,      # legacy: per-op engine cycle (overrides chunk_pattern)
    chunk_pattern="ZXYXZWYX",  # per-chunk (act2,act3) engine classes:
                            # X=(ACT,DVE) Y=(DVE,ACT) Z=(ACT,ACT) W=(DVE,DVE)
    ps_bufs=2,              # PSUM buffers for each of p1/p2
    h1_bufs=4, h2_bufs=4, h3_bufs=2,
    act1_pool_mod=2,        # every Nth act1 op runs on GPSIMD/Pool (0=off)
)


def _build(dyn_reps=1, cfg=None):
    import contextlib
    import concourse.bacc as bacc
    import concourse.mybir as mybir
    import concourse.tile as tile

    cfg = dict(DEFAULT_CFG, **(cfg or {}))
    f32 = mybir.dt.float32
    f32r = mybir.dt.float32r
    bf16 = mybir.dt.bfloat16

    nc = bacc.Bacc("TRN2", target_bir_lowering=False, debug=False)
    P = lambda name, shape, dt=f32, out=False: nc.declare_dram_parameter(
        name, shape, dt, isOutput=out)

    x_in = P("x", [C, B_PER_CORE * L], bf16)
    params = {
        "w0p": (P("w0p", [C, 160], bf16), [C, 160], bf16),
        "wbf": (P("wbf", [128, 256], bf16), [128, 256], bf16),
        "wf": (P("wf", [128, 101], f32), [128, 101], f32),
    }
    out = P("out", [H, B_PER_CORE], out=True)

    with tile.TileContext(nc) as tc:
        with (
            tc.tile_pool(name="wpool", bufs=1) as wpool,
            tc.tile_pool(name="xpool", bufs=B_PER_CORE) as xpool,
            tc.tile_pool(name="atpool", bufs=B_PER_CORE) as atpool,
            tc.tile_pool(name="btpool", bufs=B_PER_CORE) as btpool,
            tc.tile_pool(name="h1pool", bufs=cfg["h1_bufs"]) as h1pool,
            tc.tile_pool(name="h2pool", bufs=cfg["h2_bufs"]) as h2pool,
            tc.tile_pool(name="h3pool", bufs=cfg["h3_bufs"]) as h3pool,
            tc.tile_pool(name="accpool", bufs=B_PER_CORE) as accpool,
            tc.tile_pool(name="spool", bufs=1) as spool,
            tc.tile_pool(name="ps1", bufs=cfg["ps_bufs"], space="PSUM") as ps1_pool,
            tc.tile_pool(name="ps2", bufs=cfg["ps_bufs"], space="PSUM") as ps2_pool,
        ):
            reps_ctx = (tc.For_i(0, dyn_reps) if dyn_reps > 1
                        else contextlib.nullcontext())
            with reps_ctx:
                _body(nc, tc, mybir, cfg, params, x_in, out,
                      wpool, xpool, atpool, btpool, h1pool, h2pool, h3pool,
                      accpool, spool, ps1_pool, ps2_pool)

    nc.compile()
    return nc


def _body(nc, tc, mybir, cfg, params, x_in, out,
          wpool, xpool, atpool, btpool, h1pool, h2pool, h3pool,
          accpool, spool, ps1_pool, ps2_pool):
    f32 = mybir.dt.float32
    f32r = mybir.dt.float32r
    bf16 = mybir.dt.bfloat16
    AF = mybir.ActivationFunctionType
    ALU = mybir.AluOpType
    CW = TPC * L           # chunk width = 1024

    # Trigger the ACT table load (Relu/Identity set, ~1.3us) immediately so
    # it overlaps the input DMAs instead of stalling the first real act op.
    zcol = spool.tile([128, 1], f32, tag="zcol")
    nc.vector.memset(zcol[:], 0.0)
    nc.scalar.activation(zcol[:], zcol[:], AF.Relu)  # relu(0)=0; table preload

    # Params land in 4 packed DMAs (vs 17 individual ones): w0p (A/B input
    # weights), xall (all 4 batches side by side), wbf (W1bd|W2bd), wf
    # (biases + FOLD + head weights).
    w0a = wpool.tile([C, 128], bf16, tag="w0a")
    nc.sync.dma_start(w0a[:], params["w0p"][0][:, 0:128])
    w0b = wpool.tile([C, H], bf16, tag="w0b")
    nc.sync.dma_start(w0b[:], params["w0p"][0][:, 128:160])
    xbs = []
    for b in range(B_PER_CORE):
        xb = xpool.tile([C, L], bf16, tag="xb", name=f"xb{b}")
        nc.sync.dma_start(xb[:], x_in[:, b * L:(b + 1) * L])
        xbs.append(xb)
    wbf = wpool.tile([128, 256], bf16, tag="wbf")
    nc.sync.dma_start(wbf[:], params["wbf"][0][:])
    wf = wpool.tile([128, 101], f32, tag="wf")
    nc.sync.dma_start(wf[:], params["wf"][0][:])

    ld = {
        "W0a4": w0a[:], "W0b": w0b[:],
        "W1bd": wbf[:, 0:128], "W2bd": wbf[:, 128:256],
        "b04": wf[:, 0:1], "b14": wf[:, 1:2], "b24": wf[:, 2:3],
        "FOLD": wf[:, 3:35], "Wp": wf[0:H, 35:67], "bp": wf[0:H, 67:68],
        "Wo": wf[0:H, 68:100], "bo": wf[0:H, 100:101],
    }

    S4 = spool.tile([128, B_PER_CORE], f32, tag="S4")

    # Per-batch setup: AT4pb (A^T + b0, replicated x4, bf16) and BT4
    # (B^T stacked by group, bf16) stay resident for the chunk loop.
    ATs, BTs, ACCs = [], [], []
    for b in range(B_PER_CORE):
        xb = xbs[b][:]
        pA = ps1_pool.tile([128, L], f32, tag="p1")
        nc.tensor.matmul(pA[:], ld["W0a4"], xb, start=True, stop=True)
        AT4pb = atpool.tile([128, L], bf16, tag="AT4pb", name=f"AT4pb{b}")
        nc.scalar.activation(AT4pb[:], pA[:], AF.Identity, bias=ld["b04"])

        pB = ps2_pool.tile([128, TPB], f32, tag="p2")
        for g in range(NG):
            nc.tensor.matmul(
                pB[32 * g:32 * (g + 1), :], ld["W0b"],
                xb[:, TPB * g:TPB * (g + 1)].opt(),
                start=True, stop=True, tile_position=(0, 32 * g))
        BT4 = btpool.tile([128, TPB], f32, tag="BT4", name=f"BT4_{b}")
        nc.vector.tensor_copy(BT4[:], pB[:])

        acc_cols = accpool.tile([128, NCH], f32, tag="acc", name=f"acc{b}")
        ATs.append(AT4pb); BTs.append(BT4); ACCs.append(acc_cols)

    CLS = {"X": "AD", "Y": "DA", "Z": "AA", "W": "DD"}
    if cfg.get("evac_pattern") or cfg.get("act2_pattern"):
        pat2 = cfg.get("act2_pattern") or cfg["evac_pattern"]
        pat3 = cfg.get("act3_pattern") or cfg["evac_pattern"]
    else:
        cp = cfg["chunk_pattern"]
        pat2 = "".join(CLS[c][0] for c in cp)
        pat3 = "".join(CLS[c][1] for c in cp)
    g2 = g3 = 0  # per-stage chunk counters for the engine patterns

    def evac(dst, src, bias, accum=None):
        nonlocal g2, g3
        if accum is not None:
            eng = pat3[g3 % len(pat3)]
            g3 += 1
        else:
            eng = pat2[g2 % len(pat2)]
            g2 += 1
        if eng == "D":
            if accum is None:
                nc.vector.tensor_scalar(dst, src, bias, 0.0, ALU.add, ALU.max)
            else:
                # relu+bias+row-sum in one DVE op: out = (src+bias) max 0,
                # accum_out = sum(out). (tensor_scalar with accum_out would
                # repurpose op1 as the reduce op and drop the relu.)
                nc.vector.scalar_tensor_tensor(
                    dst, src, bias, zcol[:, 0:1].broadcast_to(dst.shape),
                    ALU.add, ALU.max, accum_out=accum)
        else:
            kw = {} if accum is None else {"accum_out": accum}
            nc.scalar.activation(dst, src, AF.Relu, bias=bias, **kw)

    for ch in range(NCH):
        for b in range(B_PER_CORE):
            AT4pb, BT4, acc_cols = ATs[b], BTs[b], ACCs[b]

            h1 = h1pool.tile([128, CW], bf16, tag="h1")
            for k in range(TPC):
                t = ch * TPC + k
                a1i = ch * B_PER_CORE * TPC + b * TPC + k
                pm = cfg.get("act1_pool_mod", 0)
                eng1 = nc.gpsimd if (pm and a1i % pm == pm - 1) else nc.vector
                eng1.tensor_scalar(
                    h1[:, k * L:(k + 1) * L], AT4pb[:], BT4[:, t:t + 1],
                    0.0, ALU.add, ALU.max)

            p1 = ps1_pool.tile([128, CW], f32, tag="p1")
            for m in range(CW // 512):
                nc.tensor.matmul(
                    p1[:, m * 512:(m + 1) * 512], ld["W1bd"],
                    h1[:, m * 512:(m + 1) * 512], start=True, stop=True)

            h2 = h2pool.tile([128, CW], bf16, tag="h2")
            evac(h2[:], p1[:], ld["b14"])

            p2 = ps2_pool.tile([128, CW], f32, tag="p2")
            for m in range(CW // 512):
                nc.tensor.matmul(
                    p2[:, m * 512:(m + 1) * 512], ld["W2bd"],
                    h2[:, m * 512:(m + 1) * 512], start=True, stop=True)

            if cfg.get("h3_inplace", False):
                # act3's elementwise output is dead data (only the accumulated
                # pair-sums are consumed) -- write it back over p2 in place to
                # skip the SBUF write-port penalty and the h3 SBUF tile.
                evac(p2[:], p2[:], ld["b24"], accum=acc_cols[:, ch:ch + 1])
            else:
                h3 = h3pool.tile([128, CW], bf16, tag="h3")
                evac(h3[:], p2[:], ld["b24"], accum=acc_cols[:, ch:ch + 1])

    for b in range(B_PER_CORE):
        nc.vector.tensor_reduce(
            S4[:, b:b + 1], ACCs[b][:, 0:NCH],
            axis=mybir.AxisListType.X, op=ALU.add)

    pS = ps1_pool.tile([H, B_PER_CORE], f32, tag="p1")
    nc.tensor.matmul(pS[:], ld["FOLD"], S4[:], start=True, stop=True)
    sT = spool.tile([H, B_PER_CORE], f32, tag="sT")
    nc.vector.tensor_copy(sT[:], pS[:])

    pF = ps2_pool.tile([H, B_PER_CORE], f32, tag="p2")
    nc.tensor.matmul(pF[:], ld["Wp"], sT[:], start=True, stop=True)
    fT = spool.tile([H, B_PER_CORE], f32, tag="fT")
    nc.scalar.activation(fT[:], pF[:], AF.Relu, bias=ld["bp"])

    pO = ps1_pool.tile([H, B_PER_CORE], f32, tag="p1")
    nc.tensor.matmul(pO[:], ld["Wo"], fT[:], start=True, stop=True)
    oT = spool.tile([H, B_PER_CORE], f32, tag="oT")
    nc.scalar.activation(oT[:], pO[:], AF.Identity, bias=ld["bo"])

    nc.sync.dma_start(out[:], oT[:])


def _prep_weights(W0, b0, W1, b1, W2, b2, Wp, bp, Wo, bo):
    import ml_dtypes
    bfnp = ml_dtypes.bfloat16
    W0 = np.asarray(W0, np.float32)
    W0a, W0b = W0[:C], W0[C:]
    bd = lambda W: np.kron(np.eye(NG, dtype=np.float32),
                           np.asarray(W, np.float32))
    w0p = np.concatenate(
        [np.tile(W0a, (1, NG)), W0b], axis=1).astype(bfnp)
    wbf = np.concatenate([bd(W1), bd(W2)], axis=1).astype(bfnp)
    wf = np.zeros((128, 101), np.float32)
    wf[:, 0] = np.tile(np.asarray(b0, np.float32), NG)
    wf[:, 1] = np.tile(np.asarray(b1, np.float32), NG)
    wf[:, 2] = np.tile(np.asarray(b2, np.float32), NG)
    wf[:, 3:35] = np.tile(np.eye(H, dtype=np.float32), (NG, 1))
    wf[0:H, 35:67] = np.asarray(Wp, np.float32)
    wf[0:H, 67] = np.asarray(bp, np.float32)
    wf[0:H, 68:100] = np.asarray(Wo, np.float32)
    wf[0:H, 100] = np.asarray(bo, np.float32)
    return {"w0p": np.ascontiguousarray(w0p),
            "wbf": np.ascontiguousarray(wbf), "wf": wf}


def make_in_maps(inputs):
    import ml_dtypes
    bfnp = ml_dtypes.bfloat16
    x_img = inputs["x_img"]
    wd = _prep_weights(**{k: v for k, v in inputs.items() if k != "x_img"})
    x = np.asarray(x_img, np.float32).reshape(B, C, L)
    in_maps = []
    for c in range(N_CORES):
        xc = x[c * B_PER_CORE:(c + 1) * B_PER_CORE]  # [4, C, L]
        xc = np.ascontiguousarray(
            xc.transpose(1, 0, 2).reshape(C, B_PER_CORE * L)).astype(bfnp)
        in_maps.append({"x": xc, **wd})
    return in_maps


def kernel(x_img, W0, b0, W1, b1, W2, b2, Wp, bp, Wo, bo):
    if "nc" not in _CACHE:
        _CACHE["nc"] = _build()
    nc = _CACHE["nc"]

    in_maps = make_in_maps(dict(x_img=x_img, W0=W0, b0=b0, W1=W1, b1=b1,
                                W2=W2, b2=b2, Wp=Wp, bp=bp, Wo=Wo, bo=bo))

    from concourse import bass2jax
    results = bass2jax.run_bass_via_pjrt(nc, in_maps, n_cores=N_CORES)
    full = np.concatenate([r["out"].T for r in results], axis=0)  # [32, 32]
    return full.astype(np.float32)


# revision 37
# speedup vs baseline: 4.4262x; 4.4262x over previous
"""Trainium2 Bass kernel for nn_BatchRelationalWithoutLocationsModule.

Math (per batch b, from the reference):
  o = x[b].reshape(c, h*w).T          # [L, c], c=64, L=256
  A = o @ W0[:c]; B = o @ W0[c:]      # [L, 32]
  h1_{ij} = relu(A_j + B_i + b0); h2 = relu(h1 @ W1 + b1); h3 = relu(h2 @ W2 + b2)
  s = sum_{ij} h3; out = relu(s @ Wp + bp) @ Wo + bo
  (sum is over all L^2 ordered pairs)

Distribution: pure data parallel, batch 32 -> 4 per core on 8 NeuronCores.

Device layout per core: partitions = 4 groups x 32 hidden. Group g handles
pair-row i = 64g + t. AT4pb [128, 256] = (A^T + b0) replicated over the 4
groups (bf16); BT4 [128, 64] column t = B^T[:, 64g+t] stacked by group (bf16).
Per chunk (4 t-values):
  act1 (x4): h1[:, kL:(k+1)L] = relu(AT4pb + BT4[:, t])  on DVE
             (bf16 in/out -> 4x packed mode, 2 elem read+write per port-cycle)
  mm1 (x2, N=512, bf16): p1 = W1bd^T h1   (W1bd = blockdiag(W1 x4))
  act2: h2 = relu(p1 + b14)  [128, 1024] -- ACT or DVE per balance pattern
  mm2 (x2): p2 = W2bd^T h2
  act3: h3 = relu(p2 + b24) + accum_out pair-sums -- ACT or DVE per pattern
The PSUM-sourced act2/act3 ops run at 1 elem/lane/cycle on either engine
(ACT 1.2 GHz, DVE 0.96 GHz); the balance pattern splits them ~2:1.
Then per batch reduce the chunk sums, fold the 4 groups with a 0/1 matmul,
and run the tiny head MLP on-chip.
"""
import sys
sys.path.insert(0, "/opt/trn_rl_repo")
import numpy as np

B, C, HW, L, H, NG = 32, 64, 16, 256, 32, 4
N_CORES = 8
B_PER_CORE = B // N_CORES
TPB = L // NG          # 64 t-values per batch
TPC = 4                # t-values per chunk
NCH = TPB // TPC       # 16 chunks per batch

_CACHE = {}

# evac engine pattern: 'A' = Activation engine, 'D' = DVE. Applied to the
# global sequence of act2/act3 ops round-robin.
DEFAULT_CFG = dict(
    evac_pattern=None,      # legacy: per-op engine cycle (overrides chunk_pattern)
    chunk_pattern="ZXYX",   # per-chunk (act2,act3) engine classes:
                            # X=(ACT,DVE) Y=(DVE,ACT) Z=(ACT,ACT) W=(DVE,DVE)
    ps_bufs=2,              # PSUM buffers for each of p1/p2
    h1_bufs=4, h2_bufs=4, h3_bufs=2,
    act1_pool_mod=0,        # every Nth act1 op on GPSIMD/Pool (0=off; Pool is
                            # ~25x slower than the cost model claims on HW)
)


def _build(dyn_reps=1, cfg=None):
    import contextlib
    import concourse.bacc as bacc
    import concourse.mybir as mybir
    import concourse.tile as tile

    cfg = dict(DEFAULT_CFG, **(cfg or {}))
    f32 = mybir.dt.float32
    f32r = mybir.dt.float32r
    bf16 = mybir.dt.bfloat16

    nc = bacc.Bacc("TRN2", target_bir_lowering=False, debug=False)
    P = lambda name, shape, dt=f32, out=False: nc.declare_dram_parameter(
        name, shape, dt, isOutput=out)

    x_in = P("x", [C, B_PER_CORE * L], bf16)
    params = {
        "w0p": (P("w0p", [C, 160], bf16), [C, 160], bf16),
        "wbf": (P("wbf", [128, 256], bf16), [128, 256], bf16),
        "wf": (P("wf", [128, 101], f32), [128, 101], f32),
    }
    out = P("out", [H, B_PER_CORE], out=True)

    with tile.TileContext(nc) as tc:
        with (
            tc.tile_pool(name="wpool", bufs=1) as wpool,
            tc.tile_pool(name="xpool", bufs=B_PER_CORE) as xpool,
            tc.tile_pool(name="atpool", bufs=B_PER_CORE) as atpool,
            tc.tile_pool(name="btpool", bufs=B_PER_CORE) as btpool,
            tc.tile_pool(name="h1pool", bufs=cfg["h1_bufs"]) as h1pool,
            tc.tile_pool(name="h2pool", bufs=cfg["h2_bufs"]) as h2pool,
            tc.tile_pool(name="h3pool", bufs=cfg["h3_bufs"]) as h3pool,
            tc.tile_pool(name="accpool", bufs=B_PER_CORE) as accpool,
            tc.tile_pool(name="spool", bufs=1) as spool,
            tc.tile_pool(name="ps1", bufs=cfg["ps_bufs"], space="PSUM") as ps1_pool,
            tc.tile_pool(name="ps2", bufs=cfg["ps_bufs"], space="PSUM") as ps2_pool,
        ):
            reps_ctx = (tc.For_i(0, dyn_reps) if dyn_reps > 1
                        else contextlib.nullcontext())
            with reps_ctx:
                _body(nc, tc, mybir, cfg, params, x_in, out,
                      wpool, xpool, atpool, btpool, h1pool, h2pool, h3pool,
                      accpool, spool, ps1_pool, ps2_pool)

    nc.compile()
    return nc


def _body(nc, tc, mybir, cfg, params, x_in, out,
          wpool, xpool, atpool, btpool, h1pool, h2pool, h3pool,
          accpool, spool, ps1_pool, ps2_pool):
    f32 = mybir.dt.float32
    f32r = mybir.dt.float32r
    bf16 = mybir.dt.bfloat16
    AF = mybir.ActivationFunctionType
    ALU = mybir.AluOpType
    CW = TPC * L           # chunk width = 1024

    # Trigger the ACT table load (Relu/Identity set, ~1.3us) immediately so
    # it overlaps the input DMAs instead of stalling the first real act op.
    zcol = spool.tile([128, 1], f32, tag="zcol")
    nc.vector.memset(zcol[:], 0.0)
    nc.scalar.activation(zcol[:], zcol[:], AF.Relu)  # relu(0)=0; table preload

    # Params land in 4 packed DMAs (vs 17 individual ones): w0p (A/B input
    # weights), xall (all 4 batches side by side), wbf (W1bd|W2bd), wf
    # (biases + FOLD + head weights).
    w0p = wpool.tile([C, 160], bf16, tag="w0p")
    nc.sync.dma_start(w0p[:], params["w0p"][0][:])
    xall = xpool.tile([C, B_PER_CORE * L], bf16, tag="xall")
    nc.sync.dma_start(xall[:], x_in[:])
    xbs = [xall[:, b * L:(b + 1) * L] for b in range(B_PER_CORE)]
    wbf = wpool.tile([128, 256], bf16, tag="wbf")
    nc.sync.dma_start(wbf[:], params["wbf"][0][:])
    wf = wpool.tile([128, 101], f32, tag="wf")
    nc.sync.dma_start(wf[:], params["wf"][0][:])

    ld = {
        "W0a4": w0p[:, 0:128], "W0b": w0p[:, 128:160],
        "W1bd": wbf[:, 0:128], "W2bd": wbf[:, 128:256],
        "b04": wf[:, 0:1], "b14": wf[:, 1:2], "b24": wf[:, 2:3],
        "FOLD": wf[:, 3:35], "Wp": wf[0:H, 35:67], "bp": wf[0:H, 67:68],
        "Wo": wf[0:H, 68:100], "bo": wf[0:H, 100:101],
    }

    S4 = spool.tile([128, B_PER_CORE], f32, tag="S4")

    # Per-batch setup: AT4pb (A^T + b0, replicated x4, bf16) and BT4
    # (B^T stacked by group, bf16) stay resident for the chunk loop.
    ATs, BTs, ACCs = [], [], []
    for b in range(B_PER_CORE):
        xb = xbs[b]
        pA = ps1_pool.tile([128, L], f32, tag="p1")
        nc.tensor.matmul(pA[:], ld["W0a4"], xb, start=True, stop=True)
        AT4pb = atpool.tile([128, L], bf16, tag="AT4pb", name=f"AT4pb{b}")
        if cfg.get("at4_dve", False):
            nc.vector.tensor_scalar(AT4pb[:], pA[:], ld["b04"], 0.0,
                                    ALU.add, ALU.bypass)
        else:
            nc.scalar.activation(AT4pb[:], pA[:], AF.Identity, bias=ld["b04"])

        pB = ps2_pool.tile([128, TPB], f32, tag="p2")
        for g in range(NG):
            nc.tensor.matmul(
                pB[32 * g:32 * (g + 1), :], ld["W0b"],
                xb[:, TPB * g:TPB * (g + 1)].opt(),
                start=True, stop=True, tile_position=(0, 32 * g))
        BT4 = btpool.tile([128, TPB], f32, tag="BT4", name=f"BT4_{b}")
        nc.vector.tensor_copy(BT4[:], pB[:])

        acc_cols = accpool.tile([128, NCH], f32, tag="acc", name=f"acc{b}")
        ATs.append(AT4pb); BTs.append(BT4); ACCs.append(acc_cols)

    CLS = {"X": "AD", "Y": "DA", "Z": "AA", "W": "DD"}
    if cfg.get("evac_pattern") or cfg.get("act2_pattern"):
        pat2 = cfg.get("act2_pattern") or cfg["evac_pattern"]
        pat3 = cfg.get("act3_pattern") or cfg["evac_pattern"]
    else:
        cp = cfg["chunk_pattern"]
        pat2 = "".join(CLS[c][0] for c in cp)
        pat3 = "".join(CLS[c][1] for c in cp)
    g2 = g3 = 0  # per-stage chunk counters for the engine patterns

    ovr = cfg.get("evac_override", {})

    def evac(dst, src, bias, accum=None, key=None):
        nonlocal g2, g3
        if accum is not None:
            eng = pat3[g3 % len(pat3)]
            g3 += 1
        else:
            eng = pat2[g2 % len(pat2)]
            g2 += 1
        eng = ovr.get(key, eng)
        if eng == "D":
            if accum is None:
                nc.vector.tensor_scalar(dst, src, bias, 0.0, ALU.add, ALU.max)
            else:
                # relu+bias+row-sum in one DVE op: out = (src+bias) max 0,
                # accum_out = sum(out). (tensor_scalar with accum_out would
                # repurpose op1 as the reduce op and drop the relu.)
                nc.vector.scalar_tensor_tensor(
                    dst, src, bias, zcol[:, 0:1].broadcast_to(dst.shape),
                    ALU.add, ALU.max, accum_out=accum)
        else:
            kw = {} if accum is None else {"accum_out": accum}
            nc.scalar.activation(dst, src, AF.Relu, bias=bias, **kw)

    for ch in range(NCH):
        for b in range(B_PER_CORE):
            AT4pb, BT4, acc_cols = ATs[b], BTs[b], ACCs[b]

            h1 = h1pool.tile([128, CW], bf16, tag="h1")
            for k in range(TPC):
                t = ch * TPC + k
                a1i = ch * B_PER_CORE * TPC + b * TPC + k
                pm = cfg.get("act1_pool_mod", 0)
                eng1 = nc.gpsimd if (pm and a1i % pm == pm - 1) else nc.vector
                eng1.tensor_scalar(
                    h1[:, k * L:(k + 1) * L], AT4pb[:], BT4[:, t:t + 1],
                    0.0, ALU.add, ALU.max)

            p1 = ps1_pool.tile([128, CW], f32, tag="p1")
            for m in range(CW // 512):
                nc.tensor.matmul(
                    p1[:, m * 512:(m + 1) * 512], ld["W1bd"],
                    h1[:, m * 512:(m + 1) * 512], start=True, stop=True)

            h2 = h2pool.tile([128, CW], bf16, tag="h2")
            evac(h2[:], p1[:], ld["b14"], key=(2, ch, b))

            p2 = ps2_pool.tile([128, CW], f32, tag="p2")
            for m in range(CW // 512):
                nc.tensor.matmul(
                    p2[:, m * 512:(m + 1) * 512], ld["W2bd"],
                    h2[:, m * 512:(m + 1) * 512], start=True, stop=True)

            if cfg.get("h3_inplace", False):
                # act3's elementwise output is dead data (only the accumulated
                # pair-sums are consumed) -- write it back over p2 in place to
                # skip the SBUF write-port penalty and the h3 SBUF tile.
                evac(p2[:], p2[:], ld["b24"], accum=acc_cols[:, ch:ch + 1],
                     key=(3, ch, b))
            else:
                h3 = h3pool.tile([128, CW], bf16, tag="h3")
                evac(h3[:], p2[:], ld["b24"], accum=acc_cols[:, ch:ch + 1],
                     key=(3, ch, b))

    for b in range(B_PER_CORE):
        nc.vector.tensor_reduce(
            S4[:, b:b + 1], ACCs[b][:, 0:NCH],
            axis=mybir.AxisListType.X, op=ALU.add)

    pS = ps1_pool.tile([H, B_PER_CORE], f32, tag="p1")
    nc.tensor.matmul(pS[:], ld["FOLD"], S4[:], start=True, stop=True)
    sT = spool.tile([H, B_PER_CORE], f32, tag="sT")
    nc.vector.tensor_copy(sT[:], pS[:])

    pF = ps2_pool.tile([H, B_PER_CORE], f32, tag="p2")
    nc.tensor.matmul(pF[:], ld["Wp"], sT[:], start=True, stop=True)
    fT = spool.tile([H, B_PER_CORE], f32, tag="fT")
    if cfg.get("head_dve", False):
        nc.vector.tensor_scalar(fT[:], pF[:], ld["bp"], 0.0, ALU.add, ALU.max)
    else:
        nc.scalar.activation(fT[:], pF[:], AF.Relu, bias=ld["bp"])

    pO = ps1_pool.tile([H, B_PER_CORE], f32, tag="p1")
    nc.tensor.matmul(pO[:], ld["Wo"], fT[:], start=True, stop=True)
    oT = spool.tile([H, B_PER_CORE], f32, tag="oT")
    if cfg.get("head_dve", False):
        nc.vector.tensor_scalar(oT[:], pO[:], ld["bo"], 0.0,
                                ALU.add, ALU.bypass)
    else:
        nc.scalar.activation(oT[:], pO[:], AF.Identity, bias=ld["bo"])

    nc.sync.dma_start(out[:], oT[:])


def _prep_weights(W0, b0, W1, b1, W2, b2, Wp, bp, Wo, bo):
    import ml_dtypes
    bfnp = ml_dtypes.bfloat16
    W0 = np.asarray(W0, np.float32)
    W0a, W0b = W0[:C], W0[C:]
    bd = lambda W: np.kron(np.eye(NG, dtype=np.float32),
                           np.asarray(W, np.float32))
    w0p = np.concatenate(
        [np.tile(W0a, (1, NG)), W0b], axis=1).astype(bfnp)
    wbf = np.concatenate([bd(W1), bd(W2)], axis=1).astype(bfnp)
    wf = np.zeros((128, 101), np.float32)
    wf[:, 0] = np.tile(np.asarray(b0, np.float32), NG)
    wf[:, 1] = np.tile(np.asarray(b1, np.float32), NG)
    wf[:, 2] = np.tile(np.asarray(b2, np.float32), NG)
    wf[:, 3:35] = np.tile(np.eye(H, dtype=np.float32), (NG, 1))
    wf[0:H, 35:67] = np.asarray(Wp, np.float32)
    wf[0:H, 67] = np.asarray(bp, np.float32)
    wf[0:H, 68:100] = np.asarray(Wo, np.float32)
    wf[0:H, 100] = np.asarray(bo, np.float32)
    return {"w0p": np.ascontiguousarray(w0p),
            "wbf": np.ascontiguousarray(wbf), "wf": wf}


def make_in_maps(inputs):
    import ml_dtypes
    bfnp = ml_dtypes.bfloat16
    x_img = inputs["x_img"]
    wd = _prep_weights(**{k: v for k, v in inputs.items() if k != "x_img"})
    x = np.asarray(x_img, np.float32).reshape(B, C, L)
    in_maps = []
    for c in range(N_CORES):
        xc = x[c * B_PER_CORE:(c + 1) * B_PER_CORE]  # [4, C, L]
        xc = np.ascontiguousarray(
            xc.transpose(1, 0, 2).reshape(C, B_PER_CORE * L)).astype(bfnp)
        in_maps.append({"x": xc, **wd})
    return in_maps


def kernel(x_img, W0, b0, W1, b1, W2, b2, Wp, bp, Wo, bo):
    if "nc" not in _CACHE:
        _CACHE["nc"] = _build()
    nc = _CACHE["nc"]

    in_maps = make_in_maps(dict(x_img=x_img, W0=W0, b0=b0, W1=W1, b1=b1,
                                W2=W2, b2=b2, Wp=Wp, bp=bp, Wo=Wo, bo=bo))

    from concourse import bass2jax
    results = bass2jax.run_bass_via_pjrt(nc, in_maps, n_cores=N_CORES)
    full = np.concatenate([r["out"].T for r in results], axis=0)  # [32, 32]
    return full.astype(np.float32)
